# revision 39
# baseline (speedup 1.0000x reference)
"""JointAttention TRN2 Bass kernel.

Sharding: 8 cores = batch(2) x head-group(4). Each core owns one batch
element and 4 of the 16 heads (a 256-wide channel slice). Per core:
  qT/kT projections in [c, t] layout (lhsT = W stationary, rhs = xT
  moving, bf16), psum->sbuf copies on the Activation engine (q bias
  folded into the copy; k bias dropped -- softmax-row-invariant, exact),
  v projection in [t, c] layout with a bf16 [V | ones] column block per
  head, scores^T = K^T.T @ Q^T per 128-key chunk ([k, q] layout, two
  64-row tile_position quadrants), softmax exp split across three
  engines (Activation: exact Exp; DVE + GpSimd: Schraudolph int16
  bit-trick in bf16), PV with p stationary ([q, hd] output, full
  128x128 PE utilization; ones column yields the denominators),
  per-partition reciprocal + scale on DVE, attn^T via XBAR DMA
  transpose, and the output projection (row-parallel Wo slice)
  interleaved into the next attention block. The 4 partial outputs per
  batch element are summed on the host and bo is added once.
"""

import sys

import numpy as np

if "/opt/trn_rl_repo" not in sys.path:
    sys.path.insert(0, "/opt/trn_rl_repo")

import ml_dtypes

import concourse.bass as bass
import concourse.tile as tile
from concourse import bacc, mybir
from concourse.bass_utils import run_bass_kernel_spmd

F32 = mybir.dt.float32
BF16 = mybir.dt.bfloat16
I16 = mybir.dt.int16
AFT = mybir.ActivationFunctionType
ALU = mybir.AluOpType

D = 1024          # model dim
T = 2048          # query length (= self key length)
TK = 4096         # total key length (self + context)
CS = 256          # channels per core (4 heads x 64)
NH = 4            # heads per core
HD = 64           # head dim
DC = 8            # D chunks of 128
N_CORES = 8
NKC = TK // 128   # 32 key chunks

# Schraudolph exp bit trick in bf16 space: exp(s) ~ bitcast_bf16(
# int16(s * EXP_A + EXP_B)). EXP_B tuned for round-to-nearest; the
# int16 conversion truncates in CoreSim, so fold +0.5 in.
EXP_A = 128.0 / float(np.log(2.0))
EXP_B = 127.0 * 128.0 - 7.0 + 0.5

# Per key chunk the two heads' scores go to two separate psum tiles,
# each exp'd by exactly ONE engine (the tile framework serializes
# multiple readers of a tile, so sharing would chain the engines):
# head A on Activation (exact exp), head B on DVE via the bit trick
# (50% trick adds ~7e-3 rel err). GpSimd cannot touch PSUM on TRN2,
# so it only handles SBUF-side work (v-tile ones columns).


F8 = mybir.dt.float8e4
NDR = 4  # DoubleRow chunks of 256 along the contraction (d) dim


def build_nc():
    nc = bacc.Bacc(None)

    # inputs/weights come pre-split into fp8 hi+lo pairs in DoubleRow
    # k-tile layout [128, drc, ktile, *]; d = drc*256 + ktile*128 + p
    dram = {}
    for name in ("xh", "xl", "ch", "cl"):
        dram[name] = nc.declare_dram_parameter(name, [128, NDR, 2, T], F8,
                                               isOutput=False)
    for w in ("wq", "wks", "wkc", "wvs", "wvc"):
        for s in ("h", "l"):
            dram[w + s] = nc.declare_dram_parameter(
                w + s, [128, NDR, 2, CS], F8, isOutput=False)
    dram["bq"] = nc.declare_dram_parameter("bq", [CS, 1], F32, isOutput=False)
    dram["bvs"] = nc.declare_dram_parameter("bvs", [1, CS], F32, isOutput=False)
    dram["bvc"] = nc.declare_dram_parameter("bvc", [1, CS], F32, isOutput=False)
    dram["wo"] = nc.declare_dram_parameter("wo", [CS, D], BF16, isOutput=False)
    dram["out"] = nc.declare_dram_parameter("out", [T, D], F32, isOutput=True)

    with tile.TileContext(nc) as tc:
        _emit(nc, tc, dram)
    nc.compile()
    return nc


def _emit(nc, tc, dram):
    out = dram["out"]
    from contextlib import ExitStack

    ctx = ExitStack()
    with ctx:
        consts = ctx.enter_context(tc.tile_pool(name="consts", bufs=1))
        io_pool = ctx.enter_context(tc.tile_pool(name="io", bufs=1))
        qt_pool = ctx.enter_context(tc.tile_pool(name="qt", bufs=1))
        kt_pool = ctx.enter_context(tc.tile_pool(name="kt", bufs=1))
        v_pool = ctx.enter_context(tc.tile_pool(name="v", bufs=1))
        pt_pool = ctx.enter_context(tc.tile_pool(name="pt", bufs=10))
        rt_pool = ctx.enter_context(tc.tile_pool(name="rt", bufs=2))
        an_pool = ctx.enter_context(tc.tile_pool(name="an", bufs=2))
        at_pool = ctx.enter_context(tc.tile_pool(name="at", bufs=1))
        stage_pool = ctx.enter_context(tc.tile_pool(name="stage", bufs=3))
        # PSUM: ps(2) + s2(2x2) + pvA(1) + pvB(1) = 8 banks
        ps_pool = ctx.enter_context(
            tc.tile_pool(name="ps", bufs=2, space="PSUM"))
        s2_pool = ctx.enter_context(
            tc.tile_pool(name="s2", bufs=2, space="PSUM"))
        pv_pool = ctx.enter_context(
            tc.tile_pool(name="pv", bufs=1, space="PSUM"))

        # ---- constants + input slabs, ordered by first use so the DMA
        # queue feeds the projection pipeline with minimal head latency:
        # x slabs -> self weights -> c slabs -> ctx weights -> wo.
        w_sb = {}
        bv_sb = {}

        def load_w(name):
            for s in ("h", "l"):
                t = consts.tile([128, NDR, 2, CS], F8, tag=f"w_{name}{s}",
                                name=f"w_{name}{s}")
                nc.sync.dma_start(out=t, in_=dram[name + s][:, :, :, :])
                w_sb[name + s] = t

        def load_bv(name):
            t = consts.tile([128, CS], F32, tag=f"bv_{name}", name=f"bv_{name}")
            nc.sync.dma_start(out=t, in_=dram[name][:, :].to_broadcast([128, CS]))
            bv_sb[name] = t

        io_sb = {n: io_pool.tile([128, NDR, 2, T], F8, tag=n, name=f"{n}_sb")
                 for n in ("xh", "xl", "ch", "cl")}

        def load_q(name, tq):
            ts = slice(tq * 512, (tq + 1) * 512)
            nc.sync.dma_start(out=io_sb[name][:, :, :, ts],
                              in_=dram[name][:, :, :, ts])

        # t-quarter-major input DMA so the first projection groups
        # (which contract all 8 d-chunks of one t-column) unblock early.
        load_w("wks")
        load_q("xh", 0)
        load_q("xl", 0)
        load_w("wq")
        bq_sb = consts.tile([128, 2], F32, tag="bq")
        nc.sync.dma_start(out=bq_sb,
                          in_=dram["bq"].rearrange("(a p) o -> p (a o)", p=128))
        for tq in range(1, 4):
            load_q("xh", tq)
            load_q("xl", tq)
        load_w("wvs")
        load_bv("bvs")
        for tq in range(4):
            load_q("ch", tq)
            load_q("cl", tq)
        load_w("wkc")
        load_w("wvc")
        load_bv("bvc")
        wo_sb = consts.tile([128, 2, D], BF16, tag="wo")
        nc.sync.dma_start(out=wo_sb,
                          in_=dram["wo"].rearrange("(a p) f -> p a f", p=128))

        # ---- projections ----
        qT_sb = [qt_pool.tile([128, T], BF16, tag=f"qT{cc}", name=f"qT{cc}")
                 for cc in range(2)]
        kT_sb = [kt_pool.tile([128, TK], BF16, tag=f"kT{cc}", name=f"kT{cc}")
                 for cc in range(2)]
        v_sb = [v_pool.tile([128, NH, HD + 1], BF16, tag=f"v{kc}",
                            name=f"v{kc}")
                for kc in range(NKC)]

        DR = mybir.MatmulPerfMode.DoubleRow

        def dr_terms(xn, wn):
            # 3-term compensated fp8: x_hi*W_hi + x_hi*W_lo + x_lo*W_hi
            return ((io_sb[xn + "h"], w_sb[wn + "h"]),
                    (io_sb[xn + "h"], w_sb[wn + "l"]),
                    (io_sb[xn + "l"], w_sb[wn + "h"]))

        def proj_group(xn, wn, b_t, dst, coff, cc, tc4):
            # one [c, t] projection group; Activation engine does the
            # psum->sbuf copy (per-partition bias rides along for free).
            ps = ps_pool.tile([128, 512], F32, tag="ps",
                              name=f"ps_{wn}_{cc}_{tc4}")
            terms = dr_terms(xn, wn)
            for drc in range(NDR):
                for ti, (xt, wt) in enumerate(terms):
                    nc.tensor.matmul(
                        ps,
                        wt[:, drc, :, cc * 128:(cc + 1) * 128],
                        xt[:, drc, :, tc4 * 512:(tc4 + 1) * 512],
                        start=(drc == 0 and ti == 0),
                        stop=(drc == NDR - 1 and ti == 2),
                        perf_mode=DR)
            dst_ap = dst[cc][:, coff + tc4 * 512:coff + (tc4 + 1) * 512]
            bias = b_t[:, cc:cc + 1] if b_t is not None else 0.0
            nc.scalar.activation(dst_ap, ps, AFT.Identity, bias=bias)

        def proj_v(xn, wv_n, bv_n, kc_base):
            # v in [t, c] layout: [V | ones] per head (ones col feeds
            # the softmax denominators through the PV matmul).
            terms = dr_terms(xn, wv_n)
            for tc_i in range(16):
                ps = ps_pool.tile([128, 512], F32, tag="ps",
                                  name=f"ps_{wv_n}_{tc_i}")
                for drc in range(NDR):
                    for ti, (xt, wt) in enumerate(terms):
                        nc.tensor.matmul(
                            ps[:, 0:CS],
                            xt[:, drc, :, tc_i * 128:(tc_i + 1) * 128],
                            wt[:, drc, :, :],
                            start=(drc == 0 and ti == 0),
                            stop=(drc == NDR - 1 and ti == 2),
                            perf_mode=DR)
                vt = v_sb[kc_base + tc_i]
                with nc.allow_low_precision(reason="bf16 v tiles"):
                    nc.vector.tensor_add(
                        vt[:, :, 0:HD],
                        ps[:, 0:CS].rearrange("p (h x) -> p h x", h=NH),
                        bv_sb[bv_n][:, :].rearrange("p (h x) -> p h x", h=NH))
                nc.gpsimd.memset(vt[:, :, HD:HD + 1], 1.0)

        # x-only projections first, t-quarter-major, so PE work tracks
        # the input DMA arrival with minimal head latency.
        for tc4 in range(4):
            for wn, b_t, dst in (("wks", None, kT_sb), ("wq", bq_sb, qT_sb)):
                for cc in range(2):
                    proj_group("x", wn, b_t, dst, 0, cc, tc4)
        proj_v("x", "wvs", "bvs", 0)
        for tc4 in range(4):
            for cc in range(2):
                proj_group("c", "wkc", None, kT_sb, T, cc, tc4)
        proj_v("c", "wvc", "bvc", 16)

        # ---- attention + interleaved output projection ----
        attnT_sb = [at_pool.tile([128, T], BF16, tag=f"at{cc}",
                                 name=f"attnT{cc}")
                    for cc in range(2)]
        pending = []  # deferred out-proj emitters, drained 1 per kc step

        def emit_outproj(qt):
            def go():
                st = stage_pool.tile([128, D], F32, tag="stage",
                                     name=f"ostage{qt}")
                qsl = slice(qt * 128, (qt + 1) * 128)
                for fc in range(2):
                    fsl = slice(fc * 512, (fc + 1) * 512)
                    ps = ps_pool.tile([128, 512], F32, tag="ps",
                                      name=f"ops{qt}_{fc}")
                    for cc in range(2):
                        nc.tensor.matmul(
                            ps, attnT_sb[cc][:, qsl], wo_sb[:, cc, fsl],
                            start=(cc == 0), stop=(cc == 1))
                    nc.scalar.copy(st[:, fsl], ps)
                nc.sync.dma_start(out=out[qsl, :], in_=st)
            return go

        RUNWAY = 6  # key chunks of scores emitted ahead of their PV
        for qc in range(4):
            qs = slice(qc * 512, (qc + 1) * 512)
            for pair in range(2):
                pvA = pv_pool.tile([128, NH, HD + 1], F32, tag="pvA")
                pvB = pv_pool.tile([128, NH, HD + 1], F32, tag="pvB")
                pts = []
                for kc in range(NKC):
                    ks = slice(kc * 128, (kc + 1) * 128)
                    sA = s2_pool.tile([128, 512], F32, tag=f"sA{kc % 2}",
                                      bufs=1, name=f"sA{kc % 2}")
                    sB = s2_pool.tile([128, 512], F32, tag=f"sB{kc % 2}",
                                      bufs=1, name=f"sB{kc % 2}")
                    nc.tensor.matmul(
                        sA, kT_sb[pair][0:64, ks],
                        qT_sb[pair][0:64, qs], start=True, stop=True)
                    nc.tensor.matmul(
                        sB, kT_sb[pair][64:128, ks],
                        qT_sb[pair][64:128, qs], start=True, stop=True,
                        tile_position=(64, 0))
                    sl = kc % 8
                    ptA = pt_pool.tile([128, 512], BF16,
                                       tag=f"ptA{sl}", bufs=1)
                    ptB = pt_pool.tile([128, 512], BF16,
                                       tag=f"ptB{sl}", bufs=1)
                    nc.scalar.activation(ptA, sA, AFT.Exp)
                    with nc.allow_low_precision(
                            reason="schraudolph exp bit trick"):
                        nc.vector.tensor_scalar(
                            ptB.bitcast(I16), sB,
                            EXP_A, EXP_B, ALU.mult, ALU.add)
                    pts.append((ptA, ptB))
                    if kc >= RUNWAY:
                        ck = kc - RUNWAY
                        _emit_pv(nc, pts[ck], v_sb[ck], pvA, pvB, pair,
                                 start=(ck == 0), stop=False)
                    if pending and kc % 4 == 1:
                        pending.pop(0)()
                for ck in range(NKC - RUNWAY, NKC):
                    _emit_pv(nc, pts[ck], v_sb[ck], pvA, pvB, pair,
                             start=False, stop=(ck == NKC - 1))

                # epilogue: reciprocal of denominators, scale, transpose
                rt = rt_pool.tile([128, 2, NH], F32, tag="rt")
                nc.vector.reciprocal(rt[:, 0, :], pvA[:, :, HD])
                nc.vector.reciprocal(rt[:, 1, :], pvB[:, :, HD])
                an = an_pool.tile([128, NH, 128], BF16, tag="an")
                with nc.allow_low_precision(reason="bf16 attn tiles"):
                    for h2, pv in ((0, pvA), (1, pvB)):
                        for sub in range(NH):
                            nc.scalar.mul(
                                an[:, sub, h2 * HD:(h2 + 1) * HD],
                                pv[:, sub, 0:HD],
                                rt[:, h2, sub:sub + 1])
                for sub in range(NH):
                    nc.sync.dma_start_transpose(
                        out=attnT_sb[pair][:, qc * 512 + sub * 128:
                                           qc * 512 + (sub + 1) * 128],
                        in_=an[:, sub, :])
                    if qc == 3 and pair == 1:
                        # tail: launch each out-proj block as soon as
                        # its last transpose lands
                        emit_outproj(qc * 4 + sub)()
            if qc < 3:
                for qt in range(qc * 4, qc * 4 + 4):
                    pending.append(emit_outproj(qt))
        while pending:
            pending.pop(0)()


def _emit_pv(nc, pt, vt, pvA, pvB, pair, start, stop):
    # out[q, hd|den] += p[k, q].T @ [V | ones][k, :]; p is the
    # stationary operand so all 128 PE columns carry useful work.
    # The 4 sub-accumulators share one psum bank (= one zero region):
    # only sub 0 starts the group and only sub 3 stops it.
    ptA, ptB = pt
    for h2, (pv, ptX) in enumerate(((pvA, ptA), (pvB, ptB))):
        head = 2 * pair + h2
        for sub in range(NH):
            nc.tensor.matmul(
                pv[:, sub, :],
                ptX[:, sub * 128:(sub + 1) * 128],
                vt[:, head, :],
                start=start and sub == 0, stop=stop and sub == NH - 1)


_NC_CACHE = None

_F8NP = mybir.dt.np(F8)


def _dr_split(a):
    """[D, N] f32 -> (hi, lo) fp8 pair in DoubleRow layout [128, NDR, 2, N]."""
    hi = a.astype(_F8NP)
    lo = (a - hi.astype(np.float32)).astype(_F8NP)

    def lay(t):
        return np.ascontiguousarray(
            t.reshape(NDR, 2, 128, t.shape[-1]).transpose(2, 0, 1, 3))

    return lay(hi), lay(lo)


def build_in_maps(f):
    bf = ml_dtypes.bfloat16
    x, context = f["x"], f["context"]
    B = x.shape[0]

    xs = [_dr_split(np.ascontiguousarray(x[b].T)) for b in range(B)]
    cs = [_dr_split(np.ascontiguousarray(context[b].T)) for b in range(B)]

    in_maps = []
    for b in range(B):
        for hg in range(4):
            sl = slice(hg * CS, (hg + 1) * CS)
            m = {
                "xh": xs[b][0], "xl": xs[b][1],
                "ch": cs[b][0], "cl": cs[b][1],
                "bq": (f["bq"][sl] * 0.125).reshape(CS, 1).copy(),
                "bvs": f["bvs"][sl].reshape(1, CS).copy(),
                "bvc": f["bvc"][sl].reshape(1, CS).copy(),
                "wo": np.ascontiguousarray(f["Wo"][sl, :]).astype(bf),
            }
            for wn, scale in (("wq", 0.125), ("wks", 1.0), ("wkc", 1.0),
                              ("wvs", 1.0), ("wvc", 1.0)):
                W = f[wn[0].upper() + wn[1:]][:, sl] * scale
                m[wn + "h"], m[wn + "l"] = _dr_split(W)
            in_maps.append(m)
    return in_maps


def kernel(**inputs):
    global _NC_CACHE
    if _NC_CACHE is None:
        _NC_CACHE = build_nc()
    nc = _NC_CACHE

    f = {k: np.asarray(v, dtype=np.float32) for k, v in inputs.items()}
    in_maps = build_in_maps(f)

    res = run_bass_kernel_spmd(nc, in_maps, list(range(N_CORES))).results

    bo = f["bo"]
    out = np.empty((B, T, D), dtype=np.float32)
    for b in range(B):
        acc = res[b * 4 + 0]["out"].astype(np.float32).copy()
        for hg in range(1, 4):
            acc += res[b * 4 + hg]["out"]
        out[b] = acc + bo
    return out
